# revision 41
# baseline (speedup 1.0000x reference)
"""BatchHardTripletLoss on 8 Trainium2 NeuronCores.

Math (on rows sorted by label):
  e = embeddings / ||embeddings||          (row L2 norm, computed on host)
  T = e @ e.T - 4 * [label_i == label_j]   (shift baked into the matmul)
  loss_row = relu(max_j T - min_j T - 3.7)  (= relu(hard_pos - hard_neg + 0.3))
  out = mean(loss_row)

min_j T always lands on a same-label element (the -4 shift beats any s >= -1);
self (s=1) is never the min unless the row has no other positive, in which
case max_j T < 0.7 keeps the relu at zero either way.

The -4*eq shift is injected INTO the Gram matmul: rows of a 128-row tile
span <= 128 distinct (sorted) labels, so  -4*eq = Lh.T @ Rh  with
Lh[c, i] = -2*[label_i = c-th distinct label of the tile]  and
Rh[c, j] = 2*[label_j = same], both built on host in fp8.  This removes
every eq/select/subtract vector op from the device.

Sharding: rows sorted by label, 64 tiles of 128 rows, core c owns global
tiles g = 8m + c (m = 0..7).  All positives of tile g live in the column
window W(m) = [1024m - 128, 1024m + 1280)  (needs max label multiplicity
<= 129, checked at runtime).

Gram matmuls: fp8 e4m3 in DoubleRow perf mode (k=512 as 2 chunks of 256,
both operands host-packed [128, 2, *]): warm issue rate ~216 ns per
k=256 x n=512 matmul, 2x the bf16 rate.  fp8 quantization costs ~4e-4
relative error on the final loss (measured; gate is 2e-2).  A few dummy
matmuls during the DMA preamble pre-warm the PE HAM clock gate.

Mining drains each [128, 2048] PSUM tile with both engines in parallel:
the scalar engine copies the window-bearing 512-col blocks to fp16 SBUF
(which serves all window mins and that region's max), while the vector
engine max-reduces the remaining blocks straight from PSUM.  This keeps
the PSUM-release latency under the two-buffer pipeline slack.  Per-m
finale chains (maxT/minT/diff/relu) run overlapped with the last quad;
the final [8, 1] partition sum ships to the host, which adds the cores.
"""

import numpy as np
import ml_dtypes
from contextlib import ExitStack

N, D = 8192, 512
NCORES = 8
M_TILES = 8          # row tiles per core
NQ = 4               # column quads of 2048
QW = 2048
WWID = 1408          # padded positive-window width
MARGIN_C = 3.7       # 4 - 1 + MARGIN(0.3); loss = relu(maxT - minT - 3.7)
SWIL = False         # DoubleRowSwInterleave (host-interleaved weights) vs DoubleRow


def _window(m):
    """Column window [lo, hi) containing every positive of row-tile m on
    every core (global tiles g = 8m + c, c in 0..7)."""
    lo = max(0, 1024 * m - 128)
    hi = min(N, 1024 * m + 1024 + 256)
    return lo, hi


def _min_pieces(q, m):
    """W(m) ∩ quad q as [(lo, hi, slot)] in global cols; slot in {0, 1}."""
    wlo, whi = _window(m)
    qlo, qhi = q * QW, (q + 1) * QW
    a, b = max(qlo, wlo), min(qhi, whi)
    if a >= b:
        return []
    slot = 0 if a == wlo else 1
    return [(a, b, slot)]


def _win_chunks(q, m):
    """W(m) ∩ quad q split at 512-col (PSUM bank) boundaries:
    [(lo, hi)] in global cols."""
    out = []
    for (a, b, _slot) in _min_pieces(q, m):
        c = a
        while c < b:
            nxt = min(b, (c // 512 + 1) * 512)
            out.append((c, nxt))
            c = nxt
    return out


def _split(q, m):
    """Partition the quad's four 512-col blocks into the scalar-copied
    region (contiguous, contains the whole window) and the DVE-maxed
    remainder (must be a single contiguous run).
    Returns (s_lo, s_hi, d_lo, d_hi) in quad-relative cols."""
    blocks = sorted({(lo - q * QW) // 512 for (lo, hi) in _win_chunks(q, m)}
                    | {(hi - 1 - q * QW) // 512
                       for (lo, hi) in _win_chunks(q, m)})
    if not blocks:
        sb = [0, 1]
    elif len(blocks) == 1:
        b = blocks[0]
        sb = [2, 3] if b == 3 else [b, b + 1]
    else:
        sb = list(range(blocks[0], blocks[-1] + 1))
    db = [b for b in range(4) if b not in sb]
    assert db == list(range(db[0], db[0] + len(db))), (q, m, sb, db)
    return (sb[0] * 512, (sb[-1] + 1) * 512,
            db[0] * 512, (db[-1] + 1) * 512)


def _build_program():
    import concourse.bass as bass  # noqa: F401
    import concourse.bacc as bacc
    import concourse.tile as tile
    from concourse import mybir

    f8 = mybir.dt.float8e4
    f16 = mybir.dt.float16
    f32 = mybir.dt.float32
    Alu = mybir.AluOpType
    Act = mybir.ActivationFunctionType
    Ax = mybir.AxisListType
    DRS = (mybir.MatmulPerfMode.DoubleRowSwInterleave if SWIL
           else mybir.MatmulPerfMode.DoubleRow)

    nc = bacc.Bacc("TRN2", target_bir_lowering=False, debug=False,
                   num_devices=NCORES)

    embT8 = nc.dram_tensor("embT8", [D, N], f8, kind="ExternalInput").ap()
    blkT8 = nc.dram_tensor("blkT8", [128, 4096], f8, kind="ExternalInput").ap()
    lh_d = nc.dram_tensor("lh", [128, M_TILES * 128], f8,
                          kind="ExternalInput").ap()
    rh_d = nc.dram_tensor("rh", [128, M_TILES * WWID], f8,
                          kind="ExternalInput").ap()
    out = nc.dram_tensor("out", [M_TILES, 1], f32,
                         kind="ExternalOutput").ap()

    POS = 1.0e30
    NEG = -1.0e30

    with TileCtx(nc, tile) as (tc, ctx):
        persist = ctx.enter_context(tc.tile_pool(name="persist", bufs=1))
        psum = ctx.enter_context(tc.tile_pool(name="ps", bufs=2, space="PSUM"))

        ET = [persist.tile([128, 4 * QW], f8, tag=f"et{q}", name=f"et{q}")
              for q in range(NQ)]
        BLK = persist.tile([128, 4096], f8, tag="blk")
        LH = persist.tile([128, M_TILES * 128], f8, tag="lh")
        RH = persist.tile([128, M_TILES * WWID], f8, tag="rh")
        CT = [persist.tile([128, QW], f16, tag=f"ct{m}", name=f"ct{m}")
              for m in range(M_TILES)]
        maxp = persist.tile([128, M_TILES * NQ * 2], f32, tag="maxp")
        minp = persist.tile([128, M_TILES * 2], f32, tag="minp")
        maxT = persist.tile([128, M_TILES], f32, tag="maxT")
        minT = persist.tile([128, M_TILES], f32, tag="minT")
        diffs = persist.tile([128, M_TILES], f32, tag="diffs")
        relu_d = persist.tile([128, M_TILES], f32, tag="relud")
        row_loss = persist.tile([128, 1], f32, tag="rowloss")
        ones_sb = persist.tile([128, 1], f32, tag="ones")
        negm = persist.tile([128, 1], f32, tag="negm")
        out_sb = persist.tile([M_TILES, 1], f32, tag="outsb")

        dmy = persist.tile([128, 512], f8, tag="dmy")

        nc.vector.memset(dmy[:], 0.25)
        nc.vector.memset(minp[:], POS)
        nc.vector.memset(ones_sb[:], 1.0)
        nc.vector.memset(negm[:], -MARGIN_C)

        # ---------------- loads (split across both DMA queues) -------
        src = embT8.rearrange("(k p) n -> p k n", p=128)

        def load_quad(q, eng, ks=0, ke=4):
            # [128p, k, 2048j] <- embT8[k*128+p, qlo+j]
            eng.dma_start(
                out=ET[q][:].rearrange("p (k j) -> p k j", k=4)[:, ks:ke, :],
                in_=src[:, ks:ke, q * QW:(q + 1) * QW])

        nc.sync.dma_start(out=BLK[:, :2048], in_=blkT8[:, :2048])
        load_quad(0, nc.sync, 0, 2)
        nc.scalar.dma_start(out=LH[:], in_=lh_d)
        load_quad(0, nc.scalar, 2, 4)
        rhv = rh_d.rearrange("p (m w) -> p m w", m=M_TILES)
        RHV = RH[:].rearrange("p (m w) -> p m w", m=M_TILES)
        nc.sync.dma_start(out=BLK[:, 2048:], in_=blkT8[:, 2048:])
        for m in range(3):
            nc.scalar.dma_start(out=RHV[:, m:m + 1, :], in_=rhv[:, m:m + 1, :])
        load_quad(1, nc.sync)
        for m in range(3, M_TILES):
            nc.sync.dma_start(out=RHV[:, m:m + 1, :], in_=rhv[:, m:m + 1, :])

        # pre-warm the PE clock gate while DMAs land: the HAM un-throttles
        # after ~3.4us of sustained activity, so burn dummy matmuls now
        # instead of running the first ~16 real ones at half clock
        for _ in range(11):
            dps = psum.tile([128, 512], f32, tag="ps")
            nc.tensor.matmul(dps[:], lhsT=dmy[:, :128], rhs=dmy[:],
                             start=True, stop=True)

        # ---------------- Gram + mining ----------------
        for q in range(NQ):
            qlo = q * QW
            if q >= 1 and q + 1 < NQ:
                load_quad(q + 1, nc.sync)
            m_order = ([5, 6, 7, 0, 1, 2, 3, 4] if q == NQ - 1
                       else list(range(M_TILES)))
            for m in m_order:
                wlo, _ = _window(m)
                slot0 = (m * NQ + q) * 2
                s_lo, s_hi, d_lo, d_hi = _split(q, m)
                ps = psum.tile([128, QW], f32, tag="ps",
                               name=f"ps{q}_{m}")
                for kk in range(2):
                    off = m * 512 + kk * 256
                    if SWIL:
                        lhsT = BLK[:, off:off + 256].rearrange(
                            "p (r two) -> p r two", two=2)
                    else:
                        lhsT = BLK[:, off:off + 256].rearrange(
                            "p (two r) -> p two r", two=2)
                    base = ET[q][:, 2 * kk * QW:2 * (kk + 1) * QW].rearrange(
                        "p (two j) -> p two j", two=2)
                    for j in range(4):
                        nc.tensor.matmul(
                            ps[:, j * 512:(j + 1) * 512],
                            lhsT=lhsT,
                            rhs=base[:, :, j * 512:(j + 1) * 512],
                            start=(kk == 0), stop=(kk == 1),
                            perf_mode=DRS)
                    if kk == 0:
                        for (lo, hi) in _win_chunks(q, m):
                            nc.tensor.matmul(
                                ps[:, lo - qlo:hi - qlo],
                                lhsT=LH[:, m * 128:(m + 1) * 128],
                                rhs=RH[:, m * WWID + lo - wlo:
                                       m * WWID + hi - wlo],
                                start=False, stop=False,
                                skip_group_check=True)

                # parallel drain: scalar copies the window-bearing region
                # to fp16 (serves mins + its max later), DVE maxes the rest
                # straight from PSUM.  Both run concurrently, so the psum
                # buffer frees after ~max(copy, reduce) + semaphore hops.
                nc.scalar.copy(CT[m][:, s_lo:s_hi], ps[:, s_lo:s_hi])
                nc.vector.tensor_reduce(
                    out=maxp[:, slot0:slot0 + 1],
                    in_=ps[:, d_lo:d_hi], axis=Ax.X, op=Alu.max)
                nc.vector.tensor_reduce(
                    out=maxp[:, slot0 + 1:slot0 + 2],
                    in_=CT[m][:, s_lo:s_hi], axis=Ax.X, op=Alu.max)
                for (lo, hi, slot) in _min_pieces(q, m):
                    nc.vector.tensor_reduce(
                        out=minp[:, m * 2 + slot:m * 2 + slot + 1],
                        in_=CT[m][:, lo - qlo:hi - qlo],
                        axis=Ax.X, op=Alu.min)

                if q == NQ - 1:
                    # per-m finale, overlapped with the remaining q3 tiles
                    nc.vector.tensor_reduce(
                        out=maxT[:, m:m + 1],
                        in_=maxp[:].rearrange("p (m s) -> p m s",
                                              m=M_TILES)[:, m:m + 1, :],
                        axis=Ax.X, op=Alu.max)
                    nc.vector.tensor_reduce(
                        out=minT[:, m:m + 1],
                        in_=minp[:].rearrange("p (m s) -> p m s",
                                              m=M_TILES)[:, m:m + 1, :],
                        axis=Ax.X, op=Alu.min)
                    nc.vector.tensor_tensor(
                        out=diffs[:, m:m + 1], in0=maxT[:, m:m + 1],
                        in1=minT[:, m:m + 1], op=Alu.subtract)
                    nc.scalar.activation(relu_d[:, m:m + 1],
                                         diffs[:, m:m + 1],
                                         Act.Relu, bias=negm[:])

        # ---------------- finale ----------------
        ps8 = psum.tile([M_TILES, 1], f32, tag="ps")
        nc.tensor.matmul(ps8[:], lhsT=relu_d[:], rhs=ones_sb[:],
                         start=True, stop=True)
        nc.scalar.copy(out_sb[:], ps8[:])
        nc.sync.dma_start(out=out, in_=out_sb[:])

    nc.compile()
    return nc


class TileCtx:
    """contextmanager pairing TileContext with an ExitStack (pools close
    before the TileContext schedules)."""

    def __init__(self, nc, tile_mod):
        self.nc = nc
        self.tile_mod = tile_mod

    def __enter__(self):
        self.ctx = ExitStack()
        self.ctx.__enter__()
        self.tc = self.tile_mod.TileContext(self.nc)
        self.tc.__enter__()
        return self.tc, self.ctx

    def __exit__(self, *exc):
        self.ctx.__exit__(*exc)
        return self.tc.__exit__(*exc)


def _prep_inputs(embeddings, labels):
    E = np.ascontiguousarray(np.asarray(embeddings, dtype=np.float32))
    lab = np.asarray(labels).reshape(-1)
    assert E.shape == (N, D)

    order = np.argsort(lab, kind="stable")
    E_s = E[order]
    lab_s = lab[order].astype(np.int64)
    assert np.bincount(lab_s).max() <= 129, "label multiplicity > 129"

    e = E_s / np.linalg.norm(E_s, axis=1, keepdims=True)
    e8 = e.astype(ml_dtypes.float8_e4m3)
    embT8 = np.ascontiguousarray(e8.T)  # [512, 8192]

    in_maps = []
    for c in range(NCORES):
        rows = (np.arange(M_TILES)[:, None] * 1024 + c * 128
                + np.arange(128)[None, :]).reshape(-1)
        blk8 = e8[rows]  # [1024, 512]
        # weights A/B per (m, kk): A[p, r] = blk8[128m+r, 256kk+p],
        # B[p, r] = blk8[128m+r, 256kk+128+p]; SwInterleave layout is
        # il[:, 0::2] = A[:, ::-1], il[:, 1::2] = B[:, ::-1].
        w = blk8.reshape(M_TILES, 128, 2, 2, 128)       # [m, r, kk, i, p]
        w = w.transpose(4, 0, 2, 3, 1)                  # [p, m, kk, i, r]
        if SWIL:
            w = w[:, :, :, :, ::-1]                     # reverse r
            w = w.transpose(0, 1, 2, 4, 3)              # [p, m, kk, r', i]
        bT = np.ascontiguousarray(w.reshape(128, 4096))
        lh = np.zeros((128, M_TILES, 128), dtype=ml_dtypes.float8_e4m3)
        rh = np.zeros((128, M_TILES, WWID), dtype=ml_dtypes.float8_e4m3)
        for m in range(M_TILES):
            g = M_TILES * m + c
            labg = lab_s[128 * g:128 * g + 128]
            uniq, cinv = np.unique(labg, return_inverse=True)
            lh[cinv, m, np.arange(128)] = -2.0
            wlo, whi = _window(m)
            labw = lab_s[wlo:whi]
            posn = np.searchsorted(uniq, labw)
            posn_c = np.clip(posn, 0, len(uniq) - 1)
            jj = np.nonzero(uniq[posn_c] == labw)[0]
            rh[posn_c[jj], m, jj] = 2.0
        in_maps.append({
            "embT8": embT8,
            "blkT8": bT,
            "lh": np.ascontiguousarray(lh.reshape(128, M_TILES * 128)),
            "rh": np.ascontiguousarray(rh.reshape(128, M_TILES * WWID)),
        })
    return in_maps


def kernel(embeddings, labels):
    from concourse.bass_utils import run_bass_kernel_spmd

    in_maps = _prep_inputs(embeddings, labels)
    nc = _build_program()
    res = run_bass_kernel_spmd(nc, in_maps, core_ids=list(range(NCORES)))
    global LAST_RESULTS
    LAST_RESULTS = res
    total = sum(float(np.sum(r["out"])) for r in res.results)
    return np.float32(total / N)


LAST_RESULTS = None
